# revision 6
# baseline (speedup 1.0000x reference)
"""Multi-head causal self-attention on 8 Trainium2 NeuronCores (Bass/Tile), v2.

Problem: x[2,2048,1024], 16 heads, d_k=64, causal softmax, out-proj + bias.

Sharding: core c: batch b = c//4, heads 4*(c%4)..4*(c%4)+3 (tensor parallel
over heads within 4-core groups). Phase C computes partial y over full T per
core (head-pair K=128 packed out-proj) in bf16; a ReduceScatter(add) over the
4-core group scatters 512-row slices. The RS is software-pipelined: the body
consumes the PREVIOUS rep's RS output at its start and fires its own RS at
the end, so the collective overlaps the next rep's compute; an epilogue after
the rep loop emits the final consume.

Everything flows in bf16 (tolerance 2e-2 >> bf16 error). Softmax:
 - Wk is pre-scaled on the host by 184.665/8 so scores PSUM = 184.665*logit.
 - exp is split across ScalarE (true exp, scale=1/184.665) and VectorE
   (Schraudolph: int16(psum + 16250.65) bit-cast to bf16, one tensor_scalar
   op) with a greedy per-instruction load balancer.
 - Diagonal 128-blocks get mask+Schraudolph in ONE DVE tensor_tensor add
   against a host-built [128,128] tile (valid: +16250.65, masked: -32300 ->
   bf16 ~ -1e-37).
 - attn@V is chunk-serial per head with a ones-column appended to V for
   denominators; P^T slices are consumed at exact causal widths (no padding).
"""
import sys

sys.path.insert(0, "/opt/trn_rl_repo")

import numpy as np
import ml_dtypes
import concourse.bass as bass
import concourse.mybir as mybir
from concourse.bass_utils import run_bass_kernel_spmd
from concourse.tile import TileContext

FP32 = mybir.dt.float32
F32R = mybir.dt.float32r
BF16 = mybir.dt.bfloat16
I16 = mybir.dt.int16

B, T, C = 2, 2048, 1024
H, DK = 16, 64
NCORES = 8
HPC = 4
TB = T // 128        # 16 s-blocks
CB = C // 128        # 8 contraction blocks
NCH = T // 512       # 4 chunks
GROUPS = [[0, 1, 2, 3], [4, 5, 6, 7]]

AEXP = 128.0 / float(np.log(2.0))      # 184.66497
KSCALE = AEXP / 8.0                    # folded into Wk on host
ACTSCALE = 1.0 / AEXP                  # ScalarE exp scale
BCONST = 16250.40                      # Schraudolph offset (7-bit mantissa)
MASKADD = -32300.0                     # diag masked add -> bf16 ~ -1e-37

_CACHE = {}


def _split_excess_waits(nc):
    """This walrus build encodes at most ONE sync wait per instruction.
    Hoist extras onto same-engine nops placed just before."""
    ctr = 0
    for f in nc.m.functions:
        for bb in f.blocks:
            new_insts = []
            changed = False
            for inst in bb.instructions:
                si = inst.sync_info
                if si is not None and si.on_wait and len(si.on_wait) > 1:
                    waits = list(si.on_wait)
                    for w in waits[:-1]:
                        ctr += 1
                        nop = mybir.InstNoOp(
                            name=f"I-waitsplit-{ctr}", ins=[], outs=[]
                        )
                        nop.engine = inst.engine
                        nop.sync_info = mybir.SyncInfo(on_wait=[w], on_update=[])
                        new_insts.append(nop)
                        changed = True
                    inst.sync_info = mybir.SyncInfo(
                        on_wait=[waits[-1]],
                        on_update=list(si.on_update) if si.on_update else [],
                    )
                new_insts.append(inst)
            if changed:
                bb.instructions = new_insts
    return ctr


class _Sched:
    """Greedy ScalarE/VectorE load balancer (costs in ns)."""

    def __init__(self):
        self.ns = {"A": 0.0, "V": 0.0}

    def pick(self, ca, cv):
        if self.ns["A"] + ca <= self.ns["V"] + cv:
            self.ns["A"] += ca
            return "A"
        self.ns["V"] += cv
        return "V"


def _emit_body(nc, tc, q, parts="IPSEAC"):
    sch = _Sched()

    def ecopy(out, in_, w):
        if sch.pick((172.0 + w) / 1.2, (120.0 + w) / 0.96) == "A":
            nc.scalar.copy(out, in_)
        else:
            nc.vector.tensor_copy(out, in_)

    def eexp(pt_slice, ps_slice, w):
        if sch.pick((352.0 + w) / 1.2, (120.0 + w) / 0.96) == "A":
            nc.scalar.activation(pt_slice, ps_slice,
                                 mybir.ActivationFunctionType.Exp,
                                 scale=ACTSCALE)
        else:
            nc.vector.tensor_scalar_add(pt_slice.bitcast(I16), ps_slice,
                                        BCONST)

    # ---- rotated RS consume: ys(prev rep) -> y ----
    if "Y" in parts:
        _emit_ys_to_y(nc, q, ecopy)

    # ---- input DMAs (scalar queue: waits are stale cross-rep WARs) ----
    for cb in range(CB) if "I" in parts else ():
        nc.scalar.dma_start(out=q.XT[cb][:],
                            in_=q.xt[cb * 128:(cb + 1) * 128, :])

    # ---- projection chains ----
    def qk_chain(dst, nm, p, tch):
        ps = q.pj.tile([128, 512], FP32, tag="pj", bufs=2, name="pj")
        for cb in range(CB):
            nc.tensor.matmul(ps[:],
                             q.W[nm][cb][:, p * 128:(p + 1) * 128],
                             q.XT[cb][:, tch * 512:(tch + 1) * 512],
                             start=(cb == 0), stop=(cb == CB - 1))
        ecopy(dst[:, tch * 512:(tch + 1) * 512], ps[:], 512)

    def v_chain(tt):
        ps = q.pj.tile([128, 512], FP32, tag="pj", bufs=2, name="pj")
        for cb in range(CB):
            nc.tensor.matmul(ps[:, 0:256],
                             q.XT[cb][:, tt * 128:(tt + 1) * 128],
                             q.W["wv"][cb][:],
                             start=(cb == 0), stop=(cb == CB - 1))
        for h in range(HPC):
            ecopy(q.V5[tt][:, 65 * h:65 * h + 64],
                  ps[:, h * 64:(h + 1) * 64], 64)

    # QK proj for pair 0 up front
    for tch in range(NCH) if "P" in parts else ():
        qk_chain(q.QT[0], "wq", 0, tch)
    for tch in range(NCH) if "P" in parts else ():
        qk_chain(q.KT[0], "wk", 0, tch)

    fillers = []
    if "P" not in parts:
        pass
    for tt in range(0, 8) if "P" in parts else ():
        fillers.append(lambda tt=tt: v_chain(tt))
    for tch in range(NCH) if "P" in parts else ():
        fillers.append(lambda tch=tch: qk_chain(q.QT[1], "wq", 1, tch))
    for tch in range(NCH) if "P" in parts else ():
        fillers.append(lambda tch=tch: qk_chain(q.KT[1], "wk", 1, tch))
    for tt in range(8, TB) if "P" in parts else ():
        fillers.append(lambda tt=tt: v_chain(tt))
    fill_total = len(fillers)
    fill_done = [0]

    av_q = []

    def drain_av(n):
        k = min(n, len(av_q))
        for _ in range(k):
            av_q.pop(0)()

    def av_chunk_items(p, hl, c, pts):
        """Closures: AV matmuls for chunk c of head (p, hl), then the close
        (reciprocal of the ones-row sums, broadcast via PE, divide, store
        into OUTT), then nothing else. h = global-local head index."""
        h = 2 * p + hl
        cell = {}
        items = []

        def mk_mm(jj):
            def f():
                if "ps" not in cell:
                    cell["ps"] = q.psa.tile([65, 512], FP32, tag="av", bufs=2, name="av")
                ps = cell["ps"]
                pt = pts[(hl, jj)]
                qoff = jj - 4 * c
                if qoff <= 0:
                    rhs = pt[:, 512 * c - 128 * jj:512 * (c + 1) - 128 * jj]
                    out = ps[:, 0:512]
                else:
                    wv = 512 - 128 * qoff
                    rhs = pt[:, 0:wv]
                    out = ps[:, 128 * qoff:512]
                nc.tensor.matmul(out, q.V5[jj][:, 65 * h:65 * h + 65], rhs,
                                 start=(jj == 0), stop=(jj == 4 * c + 3),
                                 skip_group_check=True)
            return f

        for jj in range(0, 4 * c + 4):
            items.append(mk_mm(jj))

        def close():
            ps = cell["ps"]
            rec = q.prc.tile([128, 512], F32R, tag="rec", bufs=2, name="rec")
            nc.vector.reciprocal(rec[64:65, :], ps[64:65, :])
            sch.ns["V"] += (120 + 512) / 0.96
            pj = q.pj.tile([128, 512], FP32, tag="pj", bufs=2, name="pj")
            nc.tensor.matmul(pj[0:64, :],
                             q.ones64[64:65, 0:64].bitcast(F32R),
                             rec[64:65, :], start=True, stop=True)
            cp = q.pcp.tile([64, 512], FP32, tag="cp", bufs=2, name="cp")
            ecopy(cp[:], ps[0:64, :], 512)
            if hl == 0:
                outsl = q.OUTT[p][0:64, 512 * c:512 * (c + 1)]
                nc.vector.tensor_tensor(out=outsl, in0=cp[:], in1=pj[0:64, :],
                                        op=mybir.AluOpType.mult)
                sch.ns["V"] += (120 + 512) / 0.96
            else:
                stg = q.pbs.tile([64, 512], BF16, tag="bst", bufs=2, name="bst")
                nc.vector.tensor_tensor(out=stg[:], in0=cp[:], in1=pj[0:64, :],
                                        op=mybir.AluOpType.mult)
                sch.ns["V"] += (120 + 512) / 0.96
                nc.sync.dma_start(
                    out=q.OUTT[p][64:128, 512 * c:512 * (c + 1)], in_=stg[:])
        items.append(close)
        return items

    def c_group(tblk, dc):
        def f():
            ps = q.pj.tile([128, 512], FP32, tag="pj", bufs=2, name="pj")
            for p2 in (0, 1):
                nc.tensor.matmul(ps[:],
                                 q.OUTT[p2][:, tblk * 128:(tblk + 1) * 128],
                                 q.WOT[p2][:, dc * 512:(dc + 1) * 512],
                                 start=(p2 == 0), stop=False)
            nc.tensor.matmul(ps[:], q.ones1[0:1, :].bitcast(F32R),
                             q.bo[0:1, dc * 512:(dc + 1) * 512],
                             start=False, stop=True)
            st = q.pys.tile([128, 512], BF16, tag="yps", bufs=3, name="yps")
            ecopy(st[:], ps[:], 512)
            nc.sync.dma_start(
                out=q.yp[tblk * 128:(tblk + 1) * 128,
                         dc * 512:(dc + 1) * 512], in_=st[:])
        return f

    # ---- attention pair drivers ----
    for p in (0, 1):
        drain_av(len(av_q))          # pair-0 chunk-3 tail before PT reuse
        pts = {}
        for jj in range(TB):
            span = T - 128 * jj
            nrounds = (span + 1023) // 1024
            for r in range(nrounds):
                w = min(1024, span - r * 1024)
                t0 = 128 * jj + r * 1024
                if r == 0:
                    for hl in (0, 1):
                        pts[(hl, jj)] = q.ppt.tile(
                            [128, span], BF16, tag=f"pt{hl}_{jj}",
                            name=f"pt{hl}_{jj}")
                psr = {}
                for hl in (0, 1):
                    psr[hl] = q.psc.tile([128, 1024], FP32, tag=f"ss{hl}",
                                         name=f"ss{hl}")
                col = 0
                while col < w and "S" in parts:
                    wn = min(512, w - col)
                    for hl in (0, 1):
                        r0 = hl * 64
                        nc.tensor.matmul(
                            psr[hl][:, col:col + wn],
                            q.KT[p][r0:r0 + 64, jj * 128:(jj + 1) * 128],
                            q.QT[p][r0:r0 + 64, t0 + col:t0 + col + wn],
                            start=True, stop=True)
                    col += wn
                for hl in (0, 1) if "E" in parts else ():
                    pt = pts[(hl, jj)]
                    o0 = r * 1024
                    if r == 0:
                        nc.vector.tensor_tensor(
                            out=pt[:, 0:128].bitcast(I16),
                            in0=psr[hl][:, 0:128], in1=q.addt[:],
                            op=mybir.AluOpType.add)
                        sch.ns["V"] += (120 + 128) / 0.96
                        if w > 128:
                            eexp(pt[:, 128:w], psr[hl][:, 128:w], w - 128)
                    else:
                        eexp(pt[:, o0:o0 + w], psr[hl][:, 0:w], w)
                drain_av(3)
            if jj % 4 == 3:
                cc = jj // 4
                if "A" in parts:
                    for hl in (0, 1):
                        av_q.extend(av_chunk_items(p, hl, cc, pts))
                if p == 1 and "C" in parts:
                    for tblk in range(4 * cc, 4 * cc + 4):
                        for dc in range(2):
                            av_q.append(c_group(tblk, dc))
            if p == 0:
                target = (fill_total * (jj + 1) + 13) // 14
                while fill_done[0] < min(fill_total, target):
                    fillers[fill_done[0]]()
                    fill_done[0] += 1

    drain_av(len(av_q))

    # ---- fire the ReduceScatter (consumed by the NEXT rep / epilogue) ----
    if "R" in parts:
        nc.gpsimd.collective_compute(
            "ReduceScatter", mybir.AluOpType.add,
            ins=[q.yp[:]], outs=[q.ys[:]], replica_groups=GROUPS)

    return sch


def _emit_ys_to_y(nc, q, ecopy):
    for i in range(4):
        nc.sync.dma_start(out=q.y[i * 128:(i + 1) * 128, :],
                          in_=q.ys[i * 128:(i + 1) * 128, :])


def build_program(n_reps=1, loop_always=False, parts="IPSEAC"):
    nc = bass.Bass("TRN2", target_bir_lowering=False, debug=False,
                   num_devices=NCORES)

    class Q:
        pass

    q = Q()
    q.xt = nc.declare_dram_parameter("xt", [C, T], BF16, isOutput=False)
    q.wdram = {
        nm: nc.declare_dram_parameter(nm, [C, HPC * DK], BF16, isOutput=False)
        for nm in ("wq", "wk", "wv")
    }
    q.wot = nc.declare_dram_parameter("wot", [HPC * DK, C], BF16,
                                      isOutput=False)
    q.bo4 = nc.declare_dram_parameter("bo4", [1, C], F32R, isOutput=False)
    q.addtp = nc.declare_dram_parameter("addtp", [128, 128], FP32,
                                        isOutput=False)
    q.y = nc.declare_dram_parameter("y", [T // 4, C], BF16, isOutput=True)

    q.yp = nc.dram_tensor("yp", [T, C], BF16)
    q.ys = nc.dram_tensor("ys", [T // 4, C], BF16)

    with TileContext(nc) as tc:
        with (
            tc.tile_pool(name="const", bufs=1) as pc,
            tc.tile_pool(name="xtp", bufs=1) as px,
            tc.tile_pool(name="wp", bufs=1) as pw,
            tc.tile_pool(name="wop", bufs=1) as pwo,
            tc.tile_pool(name="qk", bufs=1) as pqk,
            tc.tile_pool(name="v5p", bufs=1) as pv5,
            tc.tile_pool(name="ptp", bufs=1) as ppt,
            tc.tile_pool(name="outp", bufs=1) as pot,
            tc.tile_pool(name="recp", bufs=1) as prc,
            tc.tile_pool(name="cpp", bufs=1) as pcp,
            tc.tile_pool(name="bsp", bufs=1) as pbs,
            tc.tile_pool(name="ysp", bufs=1) as pys,
            tc.tile_pool(name="ybp", bufs=1) as pyb,
            tc.tile_pool(name="pjp", bufs=1, space="PSUM") as pj,
            tc.tile_pool(name="pscp", bufs=1, space="PSUM") as psc,
            tc.tile_pool(name="psap", bufs=1, space="PSUM") as psa,
        ):
            q.pj, q.psc, q.psa = pj, psc, psa
            q.ppt, q.prc, q.pcp, q.pbs, q.pys, q.pyb = (
                ppt, prc, pcp, pbs, pys, pyb)

            # constants (loaded once, reused every rep)
            q.addt = pc.tile([128, 128], FP32, name="addt")
            nc.sync.dma_start(out=q.addt[:], in_=q.addtp[:])
            q.ones64 = pc.tile([128, 64], FP32, name="ones64")
            nc.vector.memset(q.ones64[:], 1.0)
            q.ones1 = pc.tile([1, 128], FP32, name="ones1")
            nc.vector.memset(q.ones1[:], 1.0)
            q.bo = pc.tile([1, C], F32R, name="bo")
            nc.sync.dma_start(out=q.bo[:], in_=q.bo4[:])

            # persistent activations
            q.XT = [px.tile([128, T], BF16, name=f"xt{cb}")
                    for cb in range(CB)]
            q.W = {nm: [pw.tile([128, HPC * DK], BF16, name=f"{nm}{cb}")
                        for cb in range(CB)] for nm in ("wq", "wk", "wv")}
            q.WOT = [pwo.tile([128, C], BF16, name=f"wot{p2}")
                     for p2 in range(2)]
            q.QT = [pqk.tile([128, T], BF16, name=f"qt{p}") for p in range(2)]
            q.KT = [pqk.tile([128, T], BF16, name=f"kt{p}") for p in range(2)]
            q.V5 = [pv5.tile([128, HPC * 65], BF16, name=f"v5_{tt}")
                    for tt in range(TB)]
            for tt in range(TB):
                for h in range(HPC):
                    nc.vector.memset(q.V5[tt][:, 65 * h + 64:65 * h + 65], 1.0)
            q.OUTT = [pot.tile([128, T], BF16, name=f"outt{p}")
                      for p in range(2)]

            # weights are persistent across reps (loaded once)
            for nm in ("wq", "wk", "wv"):
                for cb in range(CB):
                    nc.scalar.dma_start(
                        out=q.W[nm][cb][:],
                        in_=q.wdram[nm][cb * 128:(cb + 1) * 128, :])
            for p2 in range(2):
                nc.scalar.dma_start(out=q.WOT[p2][:],
                                    in_=q.wot[p2 * 128:(p2 + 1) * 128, :])

            with nc.allow_low_precision(reason="bf16 kernel"):
                body_parts = parts.replace("Y", "").replace("R", "")
                if n_reps == 1 and not loop_always:
                    sch = _emit_body(nc, tc, q, body_parts)
                else:
                    with tc.For_i(0, n_reps, 1) as _i:
                        sch = _emit_body(nc, tc, q, body_parts)

                # epilogue: one ReduceScatter + consume (outside the loop --
                # collectives cannot appear inside control flow)
                if "C" in parts:
                    nc.gpsimd.collective_compute(
                        "ReduceScatter", mybir.AluOpType.add,
                        ins=[q.yp[:]], outs=[q.ys[:]], replica_groups=GROUPS)

                    def ecopy_e(out, in_, w):
                        nc.vector.tensor_copy(out, in_)
                    _emit_ys_to_y(nc, q, ecopy_e)

    _split_excess_waits(nc)
    import os
    if os.environ.get("KDBG"):
        print(f"sched ACT={sch.ns['A']/1e3:.1f}us DVE={sch.ns['V']/1e3:.1f}us",
              file=sys.stderr)
    return nc


def _bf16(a):
    return np.ascontiguousarray(np.asarray(a, np.float32)).astype(
        ml_dtypes.bfloat16)


def _make_in_maps(x, Wq, Wk, Wv, Wo, bo):
    s = np.arange(128)[:, None]
    t = np.arange(128)[None, :]
    addt = np.where(s <= t, BCONST, MASKADD).astype(np.float32)
    in_maps = []
    for c in range(NCORES):
        b, hh = c // 4, HPC * (c % 4)
        in_maps.append({
            "xt": _bf16(x[b].T),
            "wq": _bf16(np.concatenate([Wq[hh + i] for i in range(HPC)],
                                       axis=1)),
            "wk": _bf16(np.concatenate([Wk[hh + i] for i in range(HPC)],
                                       axis=1) * KSCALE),
            "wv": _bf16(np.concatenate([Wv[hh + i] for i in range(HPC)],
                                       axis=1)),
            "wot": _bf16(Wo[:, hh * DK:(hh + HPC) * DK].T),
            "bo4": np.asarray(bo / 4.0, np.float32).reshape(1, C),
            "addtp": addt,
        })
    return in_maps


def kernel(x, Wq, Wk, Wv, Wo, bo):
    x = np.asarray(x, dtype=np.float32)
    Wq = np.asarray(Wq, dtype=np.float32)
    Wk = np.asarray(Wk, dtype=np.float32)
    Wv = np.asarray(Wv, dtype=np.float32)
    Wo = np.asarray(Wo, dtype=np.float32)
    bo = np.asarray(bo, dtype=np.float32)

    if "nc" not in _CACHE:
        _CACHE["nc"] = build_program()
    nc = _CACHE["nc"]

    in_maps = _make_in_maps(x, Wq, Wk, Wv, Wo, bo)
    res = run_bass_kernel_spmd(nc, in_maps, list(range(NCORES)))

    out = np.empty((B, T, C), dtype=np.float32)
    for c in range(NCORES):
        b, r = c // 4, c % 4
        out[b, r * 512:(r + 1) * 512, :] = np.asarray(
            res.results[c]["y"]).astype(np.float32)
    return out


# revision 7
# speedup vs baseline: 1.2445x; 1.2445x over previous
"""Multi-head causal self-attention on 8 Trainium2 NeuronCores (Bass/Tile), v2.

Problem: x[2,2048,1024], 16 heads, d_k=64, causal softmax, out-proj + bias.

Sharding: core c: batch b = c//4, heads 4*(c%4)..4*(c%4)+3 (tensor parallel
over heads within 4-core groups). Phase C computes partial y over full T per
core (head-pair K=128 packed out-proj) in bf16; a ReduceScatter(add) over the
4-core group scatters 512-row slices. The RS is software-pipelined: the body
consumes the PREVIOUS rep's RS output at its start and fires its own RS at
the end, so the collective overlaps the next rep's compute; an epilogue after
the rep loop emits the final consume.

Everything flows in bf16 (tolerance 2e-2 >> bf16 error). Softmax:
 - Wk is pre-scaled on the host by 184.665/8 so scores PSUM = 184.665*logit.
 - exp is split across ScalarE (true exp, scale=1/184.665) and VectorE
   (Schraudolph: int16(psum + 16250.65) bit-cast to bf16, one tensor_scalar
   op) with a greedy per-instruction load balancer.
 - Diagonal 128-blocks get mask+Schraudolph in ONE DVE tensor_tensor add
   against a host-built [128,128] tile (valid: +16250.65, masked: -32300 ->
   bf16 ~ -1e-37).
 - attn@V is chunk-serial per head with a ones-column appended to V for
   denominators; P^T slices are consumed at exact causal widths (no padding).
"""
import sys

sys.path.insert(0, "/opt/trn_rl_repo")

import numpy as np
import ml_dtypes
import concourse.bass as bass
import concourse.mybir as mybir
from concourse.bass_utils import run_bass_kernel_spmd
from concourse.tile import TileContext

FP32 = mybir.dt.float32
F32R = mybir.dt.float32r
BF16 = mybir.dt.bfloat16
I16 = mybir.dt.int16

B, T, C = 2, 2048, 1024
H, DK = 16, 64
NCORES = 8
HPC = 4
TB = T // 128        # 16 s-blocks
CB = C // 128        # 8 contraction blocks
NCH = T // 512       # 4 chunks
GROUPS = [[0, 1, 2, 3], [4, 5, 6, 7]]

AEXP = 128.0 / float(np.log(2.0))      # 184.66497
KSCALE = AEXP / 8.0                    # folded into Wk on host
ACTSCALE = 1.0 / AEXP                  # ScalarE exp scale
BCONST = 16250.40                      # Schraudolph offset (7-bit mantissa)
MASKADD = -32300.0                     # diag masked add -> bf16 ~ -1e-37

_CACHE = {}


def _split_excess_waits(nc):
    """This walrus build encodes at most ONE sync wait per instruction.
    Hoist extras onto same-engine nops placed just before."""
    ctr = 0
    for f in nc.m.functions:
        for bb in f.blocks:
            new_insts = []
            changed = False
            for inst in bb.instructions:
                si = inst.sync_info
                if si is not None and si.on_wait and len(si.on_wait) > 1:
                    waits = list(si.on_wait)
                    for w in waits[:-1]:
                        ctr += 1
                        nop = mybir.InstNoOp(
                            name=f"I-waitsplit-{ctr}", ins=[], outs=[]
                        )
                        nop.engine = inst.engine
                        nop.sync_info = mybir.SyncInfo(on_wait=[w], on_update=[])
                        new_insts.append(nop)
                        changed = True
                    inst.sync_info = mybir.SyncInfo(
                        on_wait=[waits[-1]],
                        on_update=list(si.on_update) if si.on_update else [],
                    )
                new_insts.append(inst)
            if changed:
                bb.instructions = new_insts
    return ctr


class _Sched:
    """Greedy ScalarE/VectorE load balancer (costs in ns)."""

    def __init__(self):
        self.ns = {"A": 0.0, "V": 0.0}

    def pick(self, ca, cv):
        if self.ns["A"] + ca <= self.ns["V"] + cv:
            self.ns["A"] += ca
            return "A"
        self.ns["V"] += cv
        return "V"


def _emit_body(nc, tc, q, parts="IPSEAC"):
    sch = _Sched()

    def ecopy(out, in_, w):
        if sch.pick((172.0 + w) / 1.2, (120.0 + w) / 0.96) == "A":
            nc.scalar.copy(out, in_)
        else:
            nc.vector.tensor_copy(out, in_)

    def eexp(pt_slice, ps_slice, w):
        if sch.pick((352.0 + w) / 1.2, (120.0 + w) / 0.96) == "A":
            nc.scalar.activation(pt_slice, ps_slice,
                                 mybir.ActivationFunctionType.Exp,
                                 scale=ACTSCALE)
        else:
            nc.vector.tensor_scalar_add(pt_slice.bitcast(I16), ps_slice,
                                        BCONST)

    # ---- rotated RS consume: ys(prev rep) -> y ----
    if "Y" in parts:
        _emit_ys_to_y(nc, q, ecopy)

    # ---- input DMAs (scalar queue: waits are stale cross-rep WARs) ----
    for cb in range(CB) if "I" in parts else ():
        nc.scalar.dma_start(out=q.XT[cb][:],
                            in_=q.xt[cb * 128:(cb + 1) * 128, :])
    for nm in ("wq", "wk", "wv") if "I" in parts else ():
        for cb in range(CB):
            nc.scalar.dma_start(out=q.W[nm][cb][:],
                                in_=q.wdram[nm][cb * 128:(cb + 1) * 128, :])
    for p2 in range(2) if "I" in parts else ():
        nc.scalar.dma_start(out=q.WOT[p2][:],
                            in_=q.wot[p2 * 128:(p2 + 1) * 128, :])

    # ---- projection chains ----
    def qk_chain(dst, nm, p, tch):
        ps = q.pj.tile([128, 512], FP32, tag="pj", bufs=2, name="pj")
        for cb in range(CB):
            nc.tensor.matmul(ps[:],
                             q.W[nm][cb][:, p * 128:(p + 1) * 128],
                             q.XT[cb][:, tch * 512:(tch + 1) * 512],
                             start=(cb == 0), stop=(cb == CB - 1))
        ecopy(dst[:, tch * 512:(tch + 1) * 512], ps[:], 512)

    def v_chain(tt):
        ps = q.pj.tile([128, 512], FP32, tag="pj", bufs=2, name="pj")
        for cb in range(CB):
            nc.tensor.matmul(ps[:, 0:256],
                             q.XT[cb][:, tt * 128:(tt + 1) * 128],
                             q.W["wv"][cb][:],
                             start=(cb == 0), stop=(cb == CB - 1))
        for h in range(HPC):
            ecopy(q.V5[tt][:, 65 * h:65 * h + 64],
                  ps[:, h * 64:(h + 1) * 64], 64)

    # QK proj for pair 0 up front
    for tch in range(NCH) if "P" in parts else ():
        qk_chain(q.QT[0], "wq", 0, tch)
    for tch in range(NCH) if "P" in parts else ():
        qk_chain(q.KT[0], "wk", 0, tch)

    fillers = []
    if "P" not in parts:
        pass
    for tt in range(0, 8) if "P" in parts else ():
        fillers.append(lambda tt=tt: v_chain(tt))
    for tch in range(NCH) if "P" in parts else ():
        fillers.append(lambda tch=tch: qk_chain(q.QT[1], "wq", 1, tch))
    for tch in range(NCH) if "P" in parts else ():
        fillers.append(lambda tch=tch: qk_chain(q.KT[1], "wk", 1, tch))
    for tt in range(8, TB) if "P" in parts else ():
        fillers.append(lambda tt=tt: v_chain(tt))
    fill_total = len(fillers)
    fill_done = [0]

    av_q = []

    def drain_av(n):
        k = min(n, len(av_q))
        for _ in range(k):
            av_q.pop(0)()

    def av_chunk_items(p, hl, c, pts):
        """Closures: AV matmuls for chunk c of head (p, hl), then the close
        (reciprocal of the ones-row sums, broadcast via PE, divide, store
        into OUTT), then nothing else. h = global-local head index."""
        h = 2 * p + hl
        cell = {}
        items = []

        def mk_mm(jj):
            def f():
                if "ps" not in cell:
                    cell["ps"] = q.psa.tile([65, 512], FP32, tag="av", bufs=2, name="av")
                ps = cell["ps"]
                pt = pts[(hl, jj)]
                qoff = jj - 4 * c
                if qoff <= 0:
                    rhs = pt[:, 512 * c - 128 * jj:512 * (c + 1) - 128 * jj]
                    out = ps[:, 0:512]
                else:
                    wv = 512 - 128 * qoff
                    rhs = pt[:, 0:wv]
                    out = ps[:, 128 * qoff:512]
                nc.tensor.matmul(out, q.V5[jj][:, 65 * h:65 * h + 65], rhs,
                                 start=(jj == 0), stop=(jj == 4 * c + 3),
                                 skip_group_check=True)
            return f

        for jj in range(0, 4 * c + 4):
            items.append(mk_mm(jj))

        def close():
            ps = cell["ps"]
            rec = q.prc.tile([128, 512], F32R, tag="rec", bufs=2, name="rec")
            nc.vector.reciprocal(rec[64:65, :], ps[64:65, :])
            sch.ns["V"] += (120 + 512) / 0.96
            pj = q.pj.tile([128, 512], FP32, tag="pj", bufs=2, name="pj")
            nc.tensor.matmul(pj[0:64, :],
                             q.ones64[64:65, 0:64].bitcast(F32R),
                             rec[64:65, :], start=True, stop=True)
            cp = q.pcp.tile([64, 512], FP32, tag="cp", bufs=2, name="cp")
            ecopy(cp[:], ps[0:64, :], 512)
            if hl == 0:
                outsl = q.OUTT[p][0:64, 512 * c:512 * (c + 1)]
                nc.vector.tensor_tensor(out=outsl, in0=cp[:], in1=pj[0:64, :],
                                        op=mybir.AluOpType.mult)
                sch.ns["V"] += (120 + 512) / 0.96
            else:
                stg = q.pbs.tile([64, 512], BF16, tag="bst", bufs=2, name="bst")
                nc.vector.tensor_tensor(out=stg[:], in0=cp[:], in1=pj[0:64, :],
                                        op=mybir.AluOpType.mult)
                sch.ns["V"] += (120 + 512) / 0.96
                nc.sync.dma_start(
                    out=q.OUTT[p][64:128, 512 * c:512 * (c + 1)], in_=stg[:])
        items.append(close)
        return items

    def c_group(tblk, dc):
        def f():
            ps = q.pj.tile([128, 512], FP32, tag="pj", bufs=2, name="pj")
            for p2 in (0, 1):
                nc.tensor.matmul(ps[:],
                                 q.OUTT[p2][:, tblk * 128:(tblk + 1) * 128],
                                 q.WOT[p2][:, dc * 512:(dc + 1) * 512],
                                 start=(p2 == 0), stop=False)
            nc.tensor.matmul(ps[:], q.ones1[0:1, :].bitcast(F32R),
                             q.bo[0:1, dc * 512:(dc + 1) * 512],
                             start=False, stop=True)
            st = q.pys.tile([128, 512], BF16, tag="yps", bufs=3, name="yps")
            ecopy(st[:], ps[:], 512)
            nc.sync.dma_start(
                out=q.yp[tblk * 128:(tblk + 1) * 128,
                         dc * 512:(dc + 1) * 512], in_=st[:])
        return f

    # ---- attention pair drivers ----
    for p in (0, 1):
        drain_av(len(av_q))          # pair-0 chunk-3 tail before PT reuse
        pts = {}
        for jj in range(TB):
            span = T - 128 * jj
            nrounds = (span + 1023) // 1024
            for r in range(nrounds):
                w = min(1024, span - r * 1024)
                t0 = 128 * jj + r * 1024
                if r == 0:
                    for hl in (0, 1):
                        pts[(hl, jj)] = q.ppt.tile(
                            [128, span], BF16, tag=f"pt{hl}_{jj}",
                            name=f"pt{hl}_{jj}")
                psr = {}
                for hl in (0, 1):
                    psr[hl] = q.psc.tile([128, 1024], FP32, tag=f"ss{hl}",
                                         name=f"ss{hl}")
                col = 0
                while col < w and "S" in parts:
                    wn = min(512, w - col)
                    for hl in (0, 1):
                        r0 = hl * 64
                        nc.tensor.matmul(
                            psr[hl][:, col:col + wn],
                            q.KT[p][r0:r0 + 64, jj * 128:(jj + 1) * 128],
                            q.QT[p][r0:r0 + 64, t0 + col:t0 + col + wn],
                            start=True, stop=True)
                    col += wn
                for hl in (0, 1) if "E" in parts else ():
                    pt = pts[(hl, jj)]
                    o0 = r * 1024
                    if r == 0:
                        nc.vector.tensor_tensor(
                            out=pt[:, 0:128].bitcast(I16),
                            in0=psr[hl][:, 0:128], in1=q.addt[:],
                            op=mybir.AluOpType.add)
                        sch.ns["V"] += (120 + 128) / 0.96
                        if w > 128:
                            eexp(pt[:, 128:w], psr[hl][:, 128:w], w - 128)
                    else:
                        eexp(pt[:, o0:o0 + w], psr[hl][:, 0:w], w)
                drain_av(3)
            if jj % 4 == 3:
                cc = jj // 4
                if "A" in parts:
                    for hl in (0, 1):
                        av_q.extend(av_chunk_items(p, hl, cc, pts))
                if p == 1 and "C" in parts:
                    for tblk in range(4 * cc, 4 * cc + 4):
                        for dc in range(2):
                            av_q.append(c_group(tblk, dc))
            if p == 0:
                target = (fill_total * (jj + 1) + 13) // 14
                while fill_done[0] < min(fill_total, target):
                    fillers[fill_done[0]]()
                    fill_done[0] += 1

    drain_av(len(av_q))

    # ---- fire the ReduceScatter (consumed by the NEXT rep / epilogue) ----
    if "R" in parts:
        nc.gpsimd.collective_compute(
            "ReduceScatter", mybir.AluOpType.add,
            ins=[q.yp[:]], outs=[q.ys[:]], replica_groups=GROUPS)

    return sch


def _emit_ys_to_y(nc, q, ecopy):
    for i in range(4):
        nc.sync.dma_start(out=q.y[i * 128:(i + 1) * 128, :],
                          in_=q.ys[i * 128:(i + 1) * 128, :])


def build_program(n_reps=1, loop_always=False, parts="IPSEAC"):
    nc = bass.Bass("TRN2", target_bir_lowering=False, debug=False,
                   num_devices=NCORES)

    class Q:
        pass

    q = Q()
    q.xt = nc.declare_dram_parameter("xt", [C, T], BF16, isOutput=False)
    q.wdram = {
        nm: nc.declare_dram_parameter(nm, [C, HPC * DK], BF16, isOutput=False)
        for nm in ("wq", "wk", "wv")
    }
    q.wot = nc.declare_dram_parameter("wot", [HPC * DK, C], BF16,
                                      isOutput=False)
    q.bo4 = nc.declare_dram_parameter("bo4", [1, C], F32R, isOutput=False)
    q.addtp = nc.declare_dram_parameter("addtp", [128, 128], FP32,
                                        isOutput=False)
    q.y = nc.declare_dram_parameter("y", [T // 4, C], BF16, isOutput=True)

    q.yp = nc.dram_tensor("yp", [T, C], BF16)
    q.ys = nc.dram_tensor("ys", [T // 4, C], BF16)

    with TileContext(nc) as tc:
        with (
            tc.tile_pool(name="const", bufs=1) as pc,
            tc.tile_pool(name="xtp", bufs=1) as px,
            tc.tile_pool(name="wp", bufs=1) as pw,
            tc.tile_pool(name="wop", bufs=1) as pwo,
            tc.tile_pool(name="qk", bufs=1) as pqk,
            tc.tile_pool(name="v5p", bufs=1) as pv5,
            tc.tile_pool(name="ptp", bufs=1) as ppt,
            tc.tile_pool(name="outp", bufs=1) as pot,
            tc.tile_pool(name="recp", bufs=1) as prc,
            tc.tile_pool(name="cpp", bufs=1) as pcp,
            tc.tile_pool(name="bsp", bufs=1) as pbs,
            tc.tile_pool(name="ysp", bufs=1) as pys,
            tc.tile_pool(name="ybp", bufs=1) as pyb,
            tc.tile_pool(name="pjp", bufs=1, space="PSUM") as pj,
            tc.tile_pool(name="pscp", bufs=1, space="PSUM") as psc,
            tc.tile_pool(name="psap", bufs=1, space="PSUM") as psa,
        ):
            q.pj, q.psc, q.psa = pj, psc, psa
            q.ppt, q.prc, q.pcp, q.pbs, q.pys, q.pyb = (
                ppt, prc, pcp, pbs, pys, pyb)

            # constants (loaded once, reused every rep)
            q.addt = pc.tile([128, 128], FP32, name="addt")
            nc.sync.dma_start(out=q.addt[:], in_=q.addtp[:])
            q.ones64 = pc.tile([128, 64], FP32, name="ones64")
            nc.vector.memset(q.ones64[:], 1.0)
            q.ones1 = pc.tile([1, 128], FP32, name="ones1")
            nc.vector.memset(q.ones1[:], 1.0)
            q.bo = pc.tile([1, C], F32R, name="bo")
            nc.sync.dma_start(out=q.bo[:], in_=q.bo4[:])

            # persistent activations
            q.XT = [px.tile([128, T], BF16, name=f"xt{cb}")
                    for cb in range(CB)]
            q.W = {nm: [pw.tile([128, HPC * DK], BF16, name=f"{nm}{cb}")
                        for cb in range(CB)] for nm in ("wq", "wk", "wv")}
            q.WOT = [pwo.tile([128, C], BF16, name=f"wot{p2}")
                     for p2 in range(2)]
            q.QT = [pqk.tile([128, T], BF16, name=f"qt{p}") for p in range(2)]
            q.KT = [pqk.tile([128, T], BF16, name=f"kt{p}") for p in range(2)]
            q.V5 = [pv5.tile([128, HPC * 65], BF16, name=f"v5_{tt}")
                    for tt in range(TB)]
            for tt in range(TB):
                for h in range(HPC):
                    nc.vector.memset(q.V5[tt][:, 65 * h + 64:65 * h + 65], 1.0)
            q.OUTT = [pot.tile([128, T], BF16, name=f"outt{p}")
                      for p in range(2)]

            with nc.allow_low_precision(reason="bf16 kernel"):
                body_parts = parts.replace("Y", "").replace("R", "")
                if n_reps == 1 and not loop_always:
                    sch = _emit_body(nc, tc, q, body_parts)
                else:
                    with tc.For_i(0, n_reps, 1) as _i:
                        sch = _emit_body(nc, tc, q, body_parts)

                # epilogue: one ReduceScatter + consume (outside the loop --
                # collectives cannot appear inside control flow)
                if "C" in parts:
                    nc.gpsimd.collective_compute(
                        "ReduceScatter", mybir.AluOpType.add,
                        ins=[q.yp[:]], outs=[q.ys[:]], replica_groups=GROUPS)

                    def ecopy_e(out, in_, w):
                        nc.vector.tensor_copy(out, in_)
                    _emit_ys_to_y(nc, q, ecopy_e)

    _split_excess_waits(nc)
    import os
    if os.environ.get("KDBG"):
        print(f"sched ACT={sch.ns['A']/1e3:.1f}us DVE={sch.ns['V']/1e3:.1f}us",
              file=sys.stderr)
    return nc


def _bf16(a):
    return np.ascontiguousarray(np.asarray(a, np.float32)).astype(
        ml_dtypes.bfloat16)


def _make_in_maps(x, Wq, Wk, Wv, Wo, bo):
    s = np.arange(128)[:, None]
    t = np.arange(128)[None, :]
    addt = np.where(s <= t, BCONST, MASKADD).astype(np.float32)
    in_maps = []
    for c in range(NCORES):
        b, hh = c // 4, HPC * (c % 4)
        in_maps.append({
            "xt": _bf16(x[b].T),
            "wq": _bf16(np.concatenate([Wq[hh + i] for i in range(HPC)],
                                       axis=1)),
            "wk": _bf16(np.concatenate([Wk[hh + i] for i in range(HPC)],
                                       axis=1) * KSCALE),
            "wv": _bf16(np.concatenate([Wv[hh + i] for i in range(HPC)],
                                       axis=1)),
            "wot": _bf16(Wo[:, hh * DK:(hh + HPC) * DK].T),
            "bo4": np.asarray(bo / 4.0, np.float32).reshape(1, C),
            "addtp": addt,
        })
    return in_maps


def kernel(x, Wq, Wk, Wv, Wo, bo):
    x = np.asarray(x, dtype=np.float32)
    Wq = np.asarray(Wq, dtype=np.float32)
    Wk = np.asarray(Wk, dtype=np.float32)
    Wv = np.asarray(Wv, dtype=np.float32)
    Wo = np.asarray(Wo, dtype=np.float32)
    bo = np.asarray(bo, dtype=np.float32)

    if "nc" not in _CACHE:
        _CACHE["nc"] = build_program()
    nc = _CACHE["nc"]

    in_maps = _make_in_maps(x, Wq, Wk, Wv, Wo, bo)
    res = run_bass_kernel_spmd(nc, in_maps, list(range(NCORES)))

    out = np.empty((B, T, C), dtype=np.float32)
    for c in range(NCORES):
        b, r = c // 4, c % 4
        out[b, r * 512:(r + 1) * 512, :] = np.asarray(
            res.results[c]["y"]).astype(np.float32)
    return out
